# revision 1
# baseline (speedup 1.0000x reference)
"""DetectiveNN Trainium2 kernel: two 2-layer bidirectional LSTM stacks.

Strategy: 6 NeuronCores, one (stack, dir) scan unit per core, 64 streams each:
  core0 rnn fwd, core1 rnn bwd, core2 rnnp-party0 fwd, core3 rnnp-party0 bwd,
  core4 rnnp-party1 fwd, core5 rnnp-party1 bwd.
Backward units receive time-reversed inputs so every core runs the same
forward-scan program (SPMD).  Everything on-device lives in a "gate
transposed" layout: the gate/feature dimension is on SBUF partitions and the
64 streams are the free dim, so h is consumed as the matmul moving operand
and produced already transposed - no per-step transposes.  The input
projection xg = Wih @ x + b is a big GEMM done on-device per layer; xg is
injected into the recurrent PSUM accumulation through an identity stationary
chunk.  L0->L1 handoff is an in-kernel pairwise AllGather.
Speaker compaction and scatter-back are host-side numpy (pure indexing).
"""

import dataclasses
import os

import ml_dtypes
import numpy as np

T, B, D, H, P = 256, 64, 1024, 512, 2
S = 64          # streams per unit
DK = 8          # Din chunks of 128 (1024/128)
HK = 4          # H chunks of 128 (512/128)
MC = 16         # gate chunks of 128 (2048/128)
NCORE = 6
BF16 = ml_dtypes.bfloat16

_CACHE = {}


def _safe_tc(tile_mod, bass_rust):
    """TileContext whose tail drain splits sem waits one per instruction
    (this walrus build rejects any Drain carrying >1 sync wait)."""
    from concourse.vector_clock import ScopedClock

    class SafeTC(tile_mod.TileContext):
        def _drain_and_barrier(self, tick_clock, wait_clock):
            drain_inst = self.nc.sync.drain()
            wait_clock.add_sem_waits(
                drain_inst.ins, ScopedClock({None: tick_clock.global_clock})
            )
            di = drain_inst.ins
            if di.sync_info is None:
                self.nc.all_engine_barrier()
                popped = self.nc._tile_sem_poison_stack.pop()
                assert popped is self._sem_poison
                self.nc.clear_and_free_semaphores(
                    list(self.sems.allocated().values())
                )
                self.nc.all_engine_barrier()
                return
            waits = list(di.sync_info.on_wait)
            ups = list(di.sync_info.on_update)
            if len(waits) > 1:
                di.sync_info = bass_rust.SyncInfo(on_wait=[waits[0]], on_update=ups)
                for w in waits[1:]:
                    d2 = self.nc.sync.drain()
                    d2.ins.sync_info = bass_rust.SyncInfo(on_wait=[w], on_update=[])
            self.nc.all_engine_barrier()
            popped = self.nc._tile_sem_poison_stack.pop()
            assert popped is self._sem_poison
            self.nc.clear_and_free_semaphores(list(self.sems.allocated().values()))
            self.nc.all_engine_barrier()

    return SafeTC


def _rev_t(ap, t_dim_in_ap, t_stride):
    """Flip the sign of the t step in an already-sliced AP (reversed read)."""
    aps = [list(p) for p in ap.ap]
    assert aps[t_dim_in_ap][0] == t_stride, (aps, t_dim_in_ap, t_stride)
    cnt = aps[t_dim_in_ap][1]
    aps[t_dim_in_ap][0] = -t_stride
    return dataclasses.replace(ap, offset=ap.offset + (cnt - 1) * t_stride, ap=aps)


def _split_waits(nc, mybir, limit=1):
    """This walrus build rejects instructions carrying more than one sync
    wait.  Spill excess waits onto no-op absorber instructions inserted just
    before the offender (same engine, same basic block -> same semantics)."""
    for f in nc.m.functions:
        for bb in f.blocks:
            il = bb.instructions
            out = []
            changed = False
            for inst in il:
                si = inst.sync_info
                if si is not None and len(si.on_wait) > limit:
                    waits = list(si.on_wait)
                    for w in waits[:-limit] if limit else waits:
                        out.append(mybir.InstNoOp(
                            name=nc.get_next_instruction_name(),
                            engine=inst.engine,
                            sync_info=mybir.SyncInfo(on_wait=[w], on_update=[]),
                            bass_nofuse=True,
                        ))
                    inst.sync_info = mybir.SyncInfo(
                        on_wait=waits[-limit:] if limit else [],
                        on_update=list(si.on_update),
                    )
                    changed = True
                out.append(inst)
            if changed:
                bb.instructions = out


def _build_ip(tc, nc, bass, mybir, ctx, wih_sb, bias_sb, xg, xT=None, gath=None,
              va=None, vb=None, t_steps=T):
    """Input projection: xg[:, t, mc, :] = Wih @ x_t + b for all tokens."""
    dt = mybir.dt
    rhs_pool = ctx.enter_context(tc.tile_pool(name="ip_rhs", bufs=2))
    ps_pool = ctx.enter_context(tc.tile_pool(name="ip_ps", bufs=2, space="PSUM"))
    st_pool = ctx.enter_context(tc.tile_pool(name="ip_st", bufs=3))

    with tc.For_i(0, t_steps, 4, staggered_reset=True) as i:
        rhs = rhs_pool.tile([128, DK, 4, S], dt.bfloat16)
        if xT is not None:
            tok = nc.snap(i * S)
            nc.sync.dma_start(
                out=rhs[:, :, :, :],
                in_=xT.rearrange("k p n -> p k n")[:, :, bass.ds(tok, 4 * S)],
            )
        else:
            # L1: gath variants: 0=slot0, 1=slot1, 2=slot0 reversed,
            # 3=slot1 reversed.  va/vb (host data) pick this core's local-time
            # source for the fwd-half / bwd-half input chunks.
            src_a = gath[bass.ds(va, 1), :, :, :, :][0].rearrange(
                "k p t j -> p k t j")[:, :, bass.ds(i, 4), :]
            nc.sync.dma_start(out=rhs[:, 0:HK, :, :], in_=src_a)
            src_b = gath[bass.ds(vb, 1), :, :, :, :][0].rearrange(
                "k p t j -> p k t j")[:, :, bass.ds(i, 4), :]
            nc.scalar.dma_start(out=rhs[:, HK : 2 * HK, :, :], in_=src_b)
        for half in range(2):
            ps = ps_pool.tile([128, 8, 4 * S], dt.float32, space="PSUM")
            for m8 in range(8):
                mc = half * 8 + m8
                for kc in range(DK):
                    nc.tensor.matmul(
                        ps[:, m8, :],
                        wih_sb[:, kc, mc, :],
                        rhs[:, kc, :, :],
                        start=(kc == 0 and m8 % 2 == 0),
                        stop=(kc == DK - 1 and m8 % 2 == 1),
                    )
            st = st_pool.tile([128, 4, 8, S], dt.bfloat16)
            for m8 in range(8):
                mc = half * 8 + m8
                nc.vector.tensor_scalar(
                    st[:, :, m8, :],
                    ps[:, m8, :].rearrange("p (t j) -> p t j", t=4),
                    bias_sb[:, mc : mc + 1],
                    None,
                    mybir.AluOpType.add,
                )
            nc.sync.dma_start(
                out=xg[:, :, half, :][:, bass.ds(i, 4), :],
                in_=st[:, :, :, :].rearrange("p t m j -> p t (m j)"),
            )


def _build_scan(tc, nc, bass, mybir, ctx, whh_sb, ident_sb, xg, hT_store,
                f32_store, t_steps=T):
    dt = mybir.dt
    A = mybir.ActivationFunctionType
    xg_pool = ctx.enter_context(tc.tile_pool(name="sc_xg", bufs=3))
    ps_pool = ctx.enter_context(tc.tile_pool(name="sc_ps", bufs=2, space="PSUM"))
    act_pool = ctx.enter_context(tc.tile_pool(name="sc_act", bufs=2))
    tmp_pool = ctx.enter_context(tc.tile_pool(name="sc_tmp", bufs=2))
    st_pool = ctx.enter_context(tc.tile_pool(name="sc_state", bufs=1))

    h_sb = st_pool.tile([128, HK, S], dt.bfloat16, name="h_state")
    c_sb = st_pool.tile([128, HK, S], dt.float32, name="c_state")
    nc.vector.memset(h_sb[:, :, :], 0.0)
    nc.vector.memset(c_sb[:, :, :], 0.0)

    with tc.For_i(0, t_steps, 1, staggered_reset=True) as t:
        xgt = xg_pool.tile([128, MC * S], dt.bfloat16)
        nc.sync.dma_start(
            out=xgt[:, :],
            in_=xg[:, bass.ds(t, 1), :, :].rearrange("p a b c -> p (a b c)"),
        )
        g = ps_pool.tile([128, MC, S], dt.float32, space="PSUM")
        for mc in range(MC):
            nc.tensor.matmul(
                g[:, mc, :],
                ident_sb[:, :],
                xgt[:, mc * S : (mc + 1) * S],
                start=(mc in (0, 8)),
                stop=False,
            )
        for mc in range(MC):
            for kc in range(HK):
                nc.tensor.matmul(
                    g[:, mc, :],
                    whh_sb[:, kc, mc, :],
                    h_sb[:, kc, :],
                    start=False,
                    stop=(kc == HK - 1 and mc in (7, 15)),
                )
        act = act_pool.tile([128, MC, S], dt.float32)
        nc.scalar.activation(act[:, 0:8, :], g[:, 0:8, :], A.Sigmoid)
        nc.scalar.activation(act[:, 8:12, :], g[:, 8:12, :], A.Tanh)
        nc.scalar.activation(act[:, 12:16, :], g[:, 12:16, :], A.Sigmoid)
        t1 = tmp_pool.tile([128, HK, S], dt.float32)
        nc.vector.tensor_mul(t1[:, :, :], act[:, 0:4, :], act[:, 8:12, :])
        t2 = tmp_pool.tile([128, HK, S], dt.float32)
        nc.vector.tensor_mul(t2[:, :, :], act[:, 4:8, :], c_sb[:, :, :])
        nc.vector.tensor_add(c_sb[:, :, :], t1[:, :, :], t2[:, :, :])
        tcv = tmp_pool.tile([128, HK, S], dt.float32)
        nc.scalar.activation(tcv[:, :, :], c_sb[:, :, :], A.Tanh)
        hf = tmp_pool.tile([128, HK, S], dt.float32)
        nc.vector.tensor_mul(hf[:, :, :], act[:, 12:16, :], tcv[:, :, :])
        nc.scalar.activation(h_sb[:, :, :], hf[:, :, :], A.Copy)
        if hT_store is not None:
            nc.scalar.dma_start(
                out=hT_store.rearrange("k p t j -> p k t j")[:, :, bass.ds(t, 1), :],
                in_=h_sb[:, :, :].rearrange("p k (t j) -> p k t j", t=1),
            )
        if f32_store is not None:
            nc.scalar.dma_start(
                out=f32_store.rearrange("k p t j -> p k t j")[:, :, bass.ds(t, 1), :],
                in_=hf[:, :, :].rearrange("p k (t j) -> p k t j", t=1),
            )


def build_nc(t_steps=T, n_cores=NCORE):
    import bass_rust
    import concourse.bass as bass
    import concourse.mybir as mybir
    from concourse import tile
    from contextlib import ExitStack

    dt = mybir.dt
    NTOK = t_steps * S
    nc = bass.Bass("TRN2", target_bir_lowering=False, debug=False,
                   num_devices=n_cores)

    xT = nc.dram_tensor("xT", [DK, 128, NTOK], dt.bfloat16, kind="ExternalInput").ap()
    wihA = nc.dram_tensor("wihA", [128, DK, MC, 128], dt.bfloat16, kind="ExternalInput").ap()
    whhA = nc.dram_tensor("whhA", [128, HK, MC, 128], dt.bfloat16, kind="ExternalInput").ap()
    biasA = nc.dram_tensor("biasA", [128, MC], dt.float32, kind="ExternalInput").ap()
    wihB = nc.dram_tensor("wihB", [128, DK, MC, 128], dt.bfloat16, kind="ExternalInput").ap()
    whhB = nc.dram_tensor("whhB", [128, HK, MC, 128], dt.bfloat16, kind="ExternalInput").ap()
    biasB = nc.dram_tensor("biasB", [128, MC], dt.float32, kind="ExternalInput").ap()
    ident = nc.dram_tensor("ident", [128, 128], dt.bfloat16, kind="ExternalInput").ap()
    flag = nc.dram_tensor("flag", [1, 2], dt.int32, kind="ExternalInput").ap()
    out_f32 = nc.dram_tensor("out_f32", [HK, 128, t_steps, S], dt.float32,
                             kind="ExternalOutput").ap()

    xg = nc.dram_tensor("xg", [128, t_steps, 2, 8 * S], dt.bfloat16).ap()
    hT0 = nc.dram_tensor("hT0", [HK, 128, t_steps, S], dt.bfloat16).ap()
    gath = nc.dram_tensor("gath", [4, HK, 128, t_steps, S], dt.bfloat16).ap()

    SafeTC = _safe_tc(tile, bass_rust)
    groups = [[2 * k, 2 * k + 1] for k in range(n_cores // 2)]

    with SafeTC(nc) as tc, ExitStack() as ctx:
        cpool = ctx.enter_context(tc.tile_pool(name="const", bufs=1))
        wihA_sb = cpool.tile([128, DK, MC, 128], dt.bfloat16, name="wihA_sb")
        whhA_sb = cpool.tile([128, HK, MC, 128], dt.bfloat16, name="whhA_sb")
        wihB_sb = cpool.tile([128, DK, MC, 128], dt.bfloat16, name="wihB_sb")
        whhB_sb = cpool.tile([128, HK, MC, 128], dt.bfloat16, name="whhB_sb")
        biasA_sb = cpool.tile([128, MC], dt.float32, name="biasA_sb")
        biasB_sb = cpool.tile([128, MC], dt.float32, name="biasB_sb")
        ident_sb = cpool.tile([128, 128], dt.bfloat16, name="ident_sb")
        flag_sb = cpool.tile([1, 2], dt.int32, name="flag_sb")
        for sb, dr in [(wihA_sb, wihA), (whhA_sb, whhA), (wihB_sb, wihB),
                       (whhB_sb, whhB), (biasA_sb, biasA), (biasB_sb, biasB),
                       (ident_sb, ident), (flag_sb, flag)]:
            nc.sync.dma_start(out=sb[...], in_=dr[...])

        tmpa = nc.alloc_registers("va_r")
        nc.regs_load(tmpa, flag_sb[0:1, 0:1])
        va = nc.snap(tmpa, donate=True, min_val=0, max_val=3)
        tmpb = nc.alloc_registers("vb_r")
        nc.regs_load(tmpb, flag_sb[0:1, 1:2])
        vb = nc.snap(tmpb, donate=True, min_val=0, max_val=3)

        with ExitStack() as phase:
            _build_ip(tc, nc, bass, mybir, phase, wihA_sb, biasA_sb, xg,
                      xT=xT, t_steps=t_steps)
        with ExitStack() as phase:
            _build_scan(tc, nc, bass, mybir, phase, whhA_sb, ident_sb, xg,
                        hT0, None, t_steps=t_steps)
        nc.gpsimd.collective_compute(
            "AllGather", mybir.AluOpType.bypass, replica_groups=groups,
            ins=[hT0[...]], outs=[gath[0:2, :, :, :, :]],
        )
        for v in range(2):
            for kc in range(HK):
                nc.sync.dma_start(
                    out=gath[2 + v, kc, :, :, :],
                    in_=gath[v, kc, :, ::-1, :],
                )
        with ExitStack() as phase:
            _build_ip(tc, nc, bass, mybir, phase, wihB_sb, biasB_sb, xg,
                      gath=gath, va=va, vb=vb, t_steps=t_steps)
        with ExitStack() as phase:
            _build_scan(tc, nc, bass, mybir, phase, whhB_sb, ident_sb, xg,
                        None, out_f32, t_steps=t_steps)
    _split_waits(nc, mybir)
    return nc


# ---------------- host-side data prep ----------------

def _lhsT_tiles(W):
    """W: (4H', Din') -> (128, Din'/128, 4H'/128, 128) [kp, kc, mc, mp] bf16."""
    M, K = W.shape
    t = W.reshape(M // 128, 128, K // 128, 128)   # [mc, mp, kc, kp]
    return np.ascontiguousarray(t.transpose(3, 2, 0, 1)).astype(BF16)


def _unit_inputs(x_unit, t_steps):
    """x_unit: (T, S, Din) fp32 local-time order -> xT (DK,128,T*S) bf16."""
    Din = x_unit.shape[2]
    xt = x_unit.reshape(t_steps * S, Din).T          # (Din, NTOK)
    xt = xt.reshape(Din // 128, 128, t_steps * S)
    return np.ascontiguousarray(xt).astype(BF16)


def _prep_inputs(inputs, t_steps=T):
    U = np.asarray(inputs["U"], np.float32)            # (T, B, D)
    qmask = np.asarray(inputs["qmask"], np.float32)    # (B, T, P)
    U_bt = U.transpose(1, 0, 2)                        # (B, T, D)
    mask = qmask > 0
    pos = np.cumsum(mask.astype(np.int64), axis=1) - 1  # (B, T, P)

    # compaction: party p stream for batch b = utterances with speaker p,
    # packed to the front, zero-padded.
    parties = np.zeros((P, B, t_steps, D), np.float32)
    b_idx, t_idx = np.nonzero(mask[:, :, 0])
    parties[0, b_idx, pos[b_idx, t_idx, 0]] = U_bt[b_idx, t_idx]
    b_idx, t_idx = np.nonzero(mask[:, :, 1])
    parties[1, b_idx, pos[b_idx, t_idx, 1]] = U_bt[b_idx, t_idx]

    # unit inputs, (T, S, D) in unit-local time
    rnn_x = U                                          # (T, B=S, D)
    units = [
        rnn_x,
        rnn_x[::-1],
        parties[0].transpose(1, 0, 2),
        parties[0].transpose(1, 0, 2)[::-1],
        parties[1].transpose(1, 0, 2),
        parties[1].transpose(1, 0, 2)[::-1],
    ]

    def wset(stack, lay, d):
        return (
            _lhsT_tiles(np.asarray(inputs[f"{stack}_Wih{lay}"][d], np.float32)),
            _lhsT_tiles(np.asarray(inputs[f"{stack}_Whh{lay}"][d], np.float32)),
            np.ascontiguousarray(
                np.asarray(inputs[f"{stack}_b{lay}"][d], np.float32)
                .reshape(MC, 128).T
            ),
        )

    stacks = ["rnn", "rnn", "rnnp", "rnnp", "rnnp", "rnnp"]
    ident = np.eye(128, dtype=BF16)
    in_maps = []
    for c in range(NCORE):
        d = c % 2
        wA = wset(stacks[c], 0, d)
        wB = wset(stacks[c], 1, d)
        in_maps.append({
            "xT": _unit_inputs(units[c], t_steps),
            "wihA": wA[0], "whhA": wA[1], "biasA": wA[2],
            "wihB": wB[0], "whhB": wB[1], "biasB": wB[2],
            "ident": ident,
            "flag": np.array([[0, 3] if d == 0 else [2, 1]], np.int32),
        })
    return in_maps, mask, pos


def _assemble(results, mask, pos, t_steps=T):
    # per-core out: (HK, 128, T, S) fp32 -> (T, S, 512) in unit-local time
    outs = []
    for c in range(NCORE):
        o = results[c]["out_f32"].reshape(H, t_steps, S).transpose(1, 2, 0)
        if c % 2 == 1:
            o = o[::-1]                                 # back to global time
        outs.append(o)
    U_s = np.concatenate([outs[0], outs[1]], axis=-1)   # (T, B, 2H)
    E = np.stack([
        np.concatenate([outs[2], outs[3]], axis=-1),
        np.concatenate([outs[4], outs[5]], axis=-1),
    ])                                                  # (P, T, B, 2H)
    E = E.transpose(0, 2, 1, 3)                         # (P, B, T, 2H)

    U_p = np.zeros((B, t_steps, 2 * H), np.float32)
    for p in range(P):
        idx = np.clip(pos[:, :, p], 0, t_steps - 1)
        gathered = np.take_along_axis(E[p], idx[:, :, None], axis=1)
        U_p = np.where(mask[:, :, p][:, :, None], gathered, U_p)
    U_p = U_p.transpose(1, 0, 2)                        # (T, B, 2H)
    return np.concatenate([U_s, U_p], axis=-1).astype(np.float32)


def _get_compiled():
    if "nc" not in _CACHE:
        _CACHE["nc"] = build_nc()
    return _CACHE["nc"]


def kernel(**inputs):
    from concourse.bass_utils import run_bass_kernel_spmd

    nc = _get_compiled()
    in_maps, mask, pos = _prep_inputs(inputs)
    trace = bool(int(os.environ.get("KERNEL_TRACE", "0")))
    res = run_bass_kernel_spmd(nc, in_maps, list(range(NCORE)), trace=trace)
    _CACHE["last_exec_time_ns"] = res.exec_time_ns
    return _assemble(res.results, mask, pos)

